# revision 19
# baseline (speedup 1.0000x reference)
# Trainium2 Bass kernel for nn_MultiHeadAttention (B=2, S=2048, D=1024, H=16).
#
# Sharding: batch+head tensor-parallel over 8 cores. Core c handles batch
# c//4 and head-group c%4 (4 heads, 256 e-dims): column-sharded wq/wk/wv,
# row-sharded wo with the partial-output sum done on the host. Each core
# only reads its batch's q/k/v (12MB fp16 -> 6MB fp8 per core) and writes
# a [2048, 1024] fp16 partial.
#
# Projections run as fp8e4 DoubleRow matmuls (2 contraction tiles of 128
# per pass, 0.5 cyc/row): q/k/v stream fp8 from the host, wq/wk (and bq)
# pre-scaled by 16 so the fp8 weights stay clear of subnormals; the 16*16
# score scaling is folded into the softmax exp scale (0.125/256).
#
# Attention stays fp16 (DoubleRow dst must start at partition 0, which
# makes the Z ones-row trick impossible in DR mode):
#   QT/KT = (e 128, 2 e-tiles, 2048) computed via DR with weight tiles
#   stationary; V in natural (token, e) layout packed [V_h | ones] per
#   (kt, head) so AV yields row sums (Z) free in psum row 64.
#   ScoresT = (k, q) per head; exp PSUM->SBUF on ACT gives P^T directly.
#   Causal-boundary blocks multiply P^T with a resident 0/1 triangle tile
#   on DVE (fp16 2x mode).
# Bias handling: K bias dropped (softmax per-query shift invariance),
#   V bias folded into the host-side output bias (out += wo @ bv),
#   Q bias added on DVE during the PSUM->SBUF copy.
# Host sums the 4 partial outputs per batch in fp32 and adds bo + wo@bv.

import numpy as np

B, S, D, H = 2, 2048, 1024, 16
DK = D // H            # 64
NC = 8                 # cores
NH = 4                 # heads per core
EL = NH * DK           # 256 local e-dims
NCH = 4                # projection token-chunks per core
CH = S // NCH          # 512
NDT = D // 128         # 8 contraction tiles
NDP = NDT // 2         # 4 DoubleRow contraction pairs
NKT = S // 128         # 16 k-tiles
NQB = S // 512         # 4 q-blocks

SKIP, PLAIN = -1, -2   # block classes (>=0 means partial-pattern index)


def _classify_mask(mask):
    """Per (kt, qj) block classification of the (S_q, S_k) mask.

    Returns cls[kt][qj] (SKIP / PLAIN / pattern idx), rng[kt][qj] live col
    range, pr[kt][qj] partial col range, and the deduped 0/1 patterns
    (list of [128, w] float16 arrays) for the partial ranges."""
    m = np.asarray(mask).reshape(S, S)              # [q, k]; 0 = masked
    liveT = (m != 0).T                              # [k, q]
    cls = [[PLAIN] * NQB for _ in range(NKT)]
    rng = [[(0, 512)] * NQB for _ in range(NKT)]
    pr = [[(0, 0)] * NQB for _ in range(NKT)]
    uniq = {}
    pats = []
    for kt in range(NKT):
        for qj in range(NQB):
            blk = liveT[kt * 128:(kt + 1) * 128, qj * 512:(qj + 1) * 512]
            if blk.all():
                cls[kt][qj] = PLAIN
            elif not blk.any():
                cls[kt][qj] = SKIP
            else:
                live_col = blk.any(axis=0)
                nz = np.nonzero(live_col)[0]
                c0, c1 = int(nz[0]), int(nz[-1]) + 1
                rng[kt][qj] = (c0, c1)
                part_col = live_col & ~blk.all(axis=0)
                pz = np.nonzero(part_col)[0]
                p0, p1 = int(pz[0]), int(pz[-1]) + 1
                pr[kt][qj] = (p0, p1)
                pat = blk[:, p0:p1].astype(np.float16)
                key = (p1 - p0, pat.tobytes())
                if key not in uniq:
                    uniq[key] = len(pats)
                    pats.append(np.ascontiguousarray(pat))
                cls[kt][qj] = uniq[key]
    return cls, rng, pr, pats


def _build_program(cls, rng, pr, pat_widths):
    import concourse.bacc as bacc
    import concourse.mybir as mybir
    from concourse.tile import TileContext

    f32 = mybir.dt.float32
    f16 = mybir.dt.float16
    f8 = mybir.dt.float8e4
    DR = mybir.MatmulPerfMode.DoubleRow
    Exp = mybir.ActivationFunctionType.Exp
    mult = mybir.AluOpType.mult
    ESC = 0.125 / 256.0    # exp scale: 1/sqrt(DK) with the 16*16 w-scale

    # pattern offsets inside the resident mask tile
    moff = []
    o = 0
    for w in pat_widths:
        moff.append(o)
        o += w
    MW = max(o, 1)

    nc = bacc.Bacc("TRN2", target_bir_lowering=False, debug=False,
                   num_devices=NC)

    qT = nc.dram_tensor("qT", [D, S], f8, kind="ExternalInput")
    kT = nc.dram_tensor("kT", [D, S], f8, kind="ExternalInput")
    vT = nc.dram_tensor("vT", [D, S], f8, kind="ExternalInput")
    w3d = nc.dram_tensor("w3", [128, 3 * NDT * EL], f8,
                         kind="ExternalInput")
    woT = nc.dram_tensor("woT", [128, 2 * D], f16, kind="ExternalInput")
    bqd = nc.dram_tensor("bq", [128, 2], f32, kind="ExternalInput")
    maskd = nc.dram_tensor("masks", [128, MW], f16, kind="ExternalInput")
    out = nc.dram_tensor("out", [S, D], f16, kind="ExternalOutput")

    # transposed-input views: [p, t, c] with t the 128-row d-block
    qT_r = qT.ap().rearrange("(t p) c -> p t c", p=128)
    kT_r = kT.ap().rearrange("(t p) c -> p t c", p=128)
    vT_r = vT.ap().rearrange("(t p) c -> p t c", p=128)
    w3_r = w3d.ap().rearrange("p (j t e) -> p j t e", j=3, t=NDT)
    woT_r = woT.ap().rearrange("p (i e) -> p i e", i=2)

    with TileContext(nc) as tc:
        with (
            tc.tile_pool(name="const", bufs=1) as constp,
            tc.tile_pool(name="per", bufs=1) as perp,
            tc.tile_pool(name="stage", bufs=6) as stagep,
            tc.tile_pool(name="pt", bufs=10) as ptp,
            tc.tile_pool(name="zz", bufs=4) as zzp,
            tc.tile_pool(name="zb", bufs=8) as zbp,
            tc.tile_pool(name="ost", bufs=6) as ostp,
            tc.tile_pool(name="psA", bufs=2, space="PSUM") as psA,
            tc.tile_pool(name="psS", bufs=2, space="PSUM") as psS,
            tc.tile_pool(name="psO", bufs=2, space="PSUM") as psO,
        ):
            # ---- constants. w3 arrives per-tensor, interleaved with the
            # input staging DMAs, so the first matmuls start early ----
            w3 = constp.tile([128, 3, NDT, EL], f8, tag="w3")
            bq_sb = constp.tile([128, 2], f32, tag="bq")
            msk = constp.tile([128, MW], f16, tag="msk")
            woT_sb = constp.tile([128, 2, D], f16, tag="wo")

            # ---- persistent activations ----
            # QT/KT/OT: [p, e-tile, token]; head h -> (rows (h%2)*64,
            # e-tile h//2)
            QT_sb = perp.tile([128, 2, S], f16, tag="QT")
            KT_sb = perp.tile([128, 2, S], f16, tag="KT")
            OT_sb = perp.tile([128, 2, S], f16, tag="OT")
            # V natural (token, e) packed per (kt, head) as
            # [ones(64, last col=1) | V_h(64)] so the AV stationary
            # [ones-col | V] is a contiguous 65-col AP and Z (the exp row
            # sum) lands in psum row 0 -- partition 0, where the DVE
            # reciprocal can read it without a staging copy.
            V_big = perp.tile([128, NKT * NH * 2 * 64], f16, tag="Vb")
            V3 = V_big[:].rearrange("p (t x) -> p t x", x=64)

            def emit_vz():
                nc.vector.memset(V3[:, 0::2, :], 0.0)
                nc.vector.memset(V3[:, 0::2, 63:64], 1.0)

            # ---- projections (fp8 DoubleRow, 2 d-tiles per pass) ----
            # generators yield after each PE quantum (~0.2-0.4us) so the
            # attention loop can drain them as PE filler between groups
            JSRC = {"q": (0, qT_r), "k": (1, kT_r), "v": (2, vT_r)}

            def stage_tensor(name, c, split=False):
                j, src_r = JSRC[name]
                lo = c * CH
                st = stagep.tile([128, NDT, CH], f8, tag="stage")
                if split:
                    # land the first two d-tiles early so the PE starts
                    # before the full chunk arrives
                    nc.sync.dma_start(out=st[:, 0:2, :],
                                      in_=src_r[:, 0:2, lo:lo + CH])
                    nc.sync.dma_start(out=st[:, 2:NDT, :],
                                      in_=src_r[:, 2:NDT, lo:lo + CH])
                else:
                    nc.sync.dma_start(out=st[:], in_=src_r[:, :, lo:lo + CH])
                return st

            def gen_qk_ep(name, c, ep, st):
                # transposed layout [e, token]; out [64 e, 512] per piece;
                # psum [128, 512] holds 2 pieces (bases 0, 64)
                w = w3[:, JSRC[name][0]]
                lo = c * CH
                ps = psA.tile([128, 512], f32, tag="proj",
                              name=f"{name}p{c}{ep}")
                for pp in range(2):
                    sub = ps[pp * 64:pp * 64 + 64, :]
                    e0 = ep * 128 + pp * 64
                    for t in range(NDP):
                        nc.tensor.matmul(
                            sub, w[:, 2 * t:2 * t + 2, e0:e0 + 64],
                            st[:, 2 * t:2 * t + 2, :],
                            start=(t == 0), stop=(t == NDP - 1),
                            perf_mode=DR)
                    yield
                if name == "q":
                    nc.vector.tensor_scalar_add(
                        QT_sb[:, ep, lo:lo + CH], ps[:], bq_sb[:, ep:ep + 1])
                else:
                    nc.vector.tensor_copy(KT_sb[:, ep, lo:lo + CH], ps[:])

            def gen_v(c, st):
                # natural layout: out [64 tok, 256 e] per subtile; a psum
                # [128, 512] holds one kt (128 tokens) per col-half
                w = w3[:, 2]
                for half in range(2):
                    ps = psA.tile([128, 512], f32, tag="proj",
                                  name=f"vp{c}{half}")
                    for pos in range(4):       # (base, col-half)
                        t0 = half * 256 + pos * 64
                        ob = (pos % 2) * 64
                        oc = (pos // 2) * 256
                        sub = ps[ob:ob + 64, oc:oc + 256]
                        for t in range(NDP):
                            nc.tensor.matmul(
                                sub, st[:, 2 * t:2 * t + 2, t0:t0 + 64],
                                w[:, 2 * t:2 * t + 2, :],
                                start=(t == 0), stop=(t == NDP - 1),
                                perf_mode=DR)
                        if pos == 1:
                            yield
                    for kk in range(2):        # kt within this psum tile
                        kt = c * 4 + half * 2 + kk
                        nc.vector.tensor_copy(
                            V3[:, kt * NH * 2 + 1:(kt + 1) * NH * 2:2, :],
                            ps[:, kk * 256:(kk + 1) * 256]
                            .rearrange("p (h e) -> p h e", h=NH))
                    yield

            def gen_chunk3(c):
                stq = stage_tensor("q", c)
                yield from gen_qk_ep("q", c, 0, stq)
                yield from gen_qk_ep("q", c, 1, stq)
                stk = stage_tensor("k", c)
                yield from gen_qk_ep("k", c, 0, stk)
                yield from gen_qk_ep("k", c, 1, stk)
                stv = stage_tensor("v", c)
                yield from gen_v(c, stv)

            def chain(*gens):
                for g in gens:
                    yield from g

            def drain(g):
                for _ in g:
                    pass

            # ---- attention ----
            # Per q-block, heads processed as two interleaved pairs so PE
            # always has the other head's matmuls while ACT runs exp. The
            # AV matmul lags scores by one group so it never waits on exp,
            # and a caller-supplied filler (projection / oproj pieces) is
            # drained between groups to keep the PE queue dense.
            def emit_attention(qj, pairs=(0, 1), defer_oproj=False,
                               tail=False, kw=False, filler=None):
                qlo = qj * 512
                acts = [kt for kt in range(NKT) if cls[kt][qj] != SKIP]
                if not acts:
                    return
                groups = [acts[gi:gi + 2] for gi in range(0, len(acts), 2)]

                def pull_filler(n):
                    for _ in range(n):
                        if filler is not None:
                            try:
                                next(filler)
                                continue
                            except StopIteration:
                                pass
                        if kw:
                            emit_keepwarm(1, 700 + qj * 100
                                          + pull_filler.kwn)
                            pull_filler.kwn += 1
                        break
                pull_filler.kwn = 0

                def emit_scores(grp, h, sc):
                    ep, hp = h // 2, (h % 2) * 64
                    for i, kt in enumerate(grp):
                        c0, c1 = rng[kt][qj]
                        klo = kt * 128
                        nc.tensor.matmul(
                            sc[:, i * 512 + c0:i * 512 + c1],
                            KT_sb[hp:hp + 64, ep, klo:klo + 128],
                            QT_sb[hp:hp + 64, ep, qlo + c0:qlo + c1],
                            start=True, stop=True)

                def emit_exp_mask(grp, pt, sc):
                    spans = [(i * 512 + rng[kt][qj][0],
                              i * 512 + rng[kt][qj][1])
                             for i, kt in enumerate(grp)]
                    lo, hi2 = spans[0][0], spans[-1][1]
                    dead = (hi2 - lo) - sum(b - a for a, b in spans)
                    exp_spans = spans if dead > 200 else [(lo, hi2)]
                    for a, bnd in exp_spans:
                        nc.scalar.activation(pt[:, a:bnd], sc[:, a:bnd],
                                             Exp, scale=ESC)
                    for i, kt in enumerate(grp):
                        cl = cls[kt][qj]
                        if cl >= 0:
                            pp0, pp1 = pr[kt][qj]
                            sl = slice(i * 512 + pp0, i * 512 + pp1)
                            nc.vector.tensor_tensor(
                                pt[:, sl], pt[:, sl],
                                msk[:, moff[cl]:moff[cl] + pp1 - pp0],
                                op=mult)

                def emit_av(grp, hi, h, pt, ots, n_done):
                    for i, kt in enumerate(grp):
                        c0, c1 = rng[kt][qj]
                        base = (kt * NH + h) * 128
                        vap = V_big[:, base + 63:base + 128]
                        n_done[hi] += 1
                        nc.tensor.matmul(
                            ots[hi][0:65, c0:c1], vap,
                            pt[:, i * 512 + c0:i * 512 + c1],
                            start=(n_done[hi] == 1),
                            stop=(n_done[hi] == len(acts)))

                for pair in pairs:
                    hs = (pair * 2, pair * 2 + 1)   # local head ids
                    ots = [psO.tile([128, 512], f32, tag="ot",
                                    name=f"ot{qj}{h}") for h in hs]
                    n_done = [0, 0]
                    pend = []   # (grp, hi, h, pt) AV lagging one group
                    for gidx, grp in enumerate(groups):
                        for hi, h in enumerate(hs):
                            sc = psS.tile([128, 1024], f32, tag="score")
                            emit_scores(grp, h, sc)
                            pt = ptp.tile([128, 1024], f16, tag="pt")
                            emit_exp_mask(grp, pt, sc)
                            if pend:
                                emit_av(*pend.pop(0), ots, n_done)
                            pend.append((grp, hi, h, pt))
                            pull_filler(1)
                            if kw:
                                emit_keepwarm(2, 50 + qj * 20 + gidx * 4
                                              + pair * 2 + hi)
                    while pend:
                        emit_av(*pend.pop(0), ots, n_done)
                    for hi, h in enumerate(hs):
                        # normalize: row 0 of ot = Z (the ones column).
                        # 65-row copy moves Z+AV to SBUF and frees the
                        # PSUM tile early; the reciprocal reads Z at
                        # partition 0 with no staging copy.
                        ep, hp = h // 2, (h % 2) * 64
                        ot = ots[hi]
                        oc = zbp.tile([65, 512], f32, tag="oc")
                        if tail and pair == pairs[-1]:
                            # ACT is idle once the last exps are done
                            nc.scalar.copy(oc[:], ot[0:65, :])
                        else:
                            nc.vector.tensor_copy(oc[:], ot[0:65, :])
                        rz = zzp.tile([1, 512], f32, tag="z")
                        nc.vector.reciprocal_approx_fast(rz[:], oc[0:1, :])
                        rb = zbp.tile([64, 512], f32, tag="zb")
                        nc.gpsimd.partition_broadcast(rb[:], rz[:],
                                                      channels=64)
                        nc.vector.tensor_tensor(
                            OT_sb[hp:hp + 64, ep, qlo:qlo + 512],
                            oc[1:65, :], rb[:], op=mult)
                if not defer_oproj:
                    emit_oproj_qblock(qj)

            # ---- output projection (partial over local e-dims) ----
            # generator yielding per half-tile; psA ring (transient, like
            # the projection pieces it interleaves with)
            def gen_oproj(qj, act_copy=False):
                for g in range(qj * 4, (qj + 1) * 4):
                    osr = ostp.tile([128, D], f16, tag="ost")
                    for jh in range(2):
                        po = psA.tile([128, 512], f32, tag="proj",
                                      name=f"po{g}{jh}")
                        for ep in range(2):
                            nc.tensor.matmul(
                                po[:], OT_sb[:, ep, g * 128:(g + 1) * 128],
                                woT_sb[:, ep, jh * 512:(jh + 1) * 512],
                                start=(ep == 0), stop=(ep == 1))
                        if act_copy and jh == 0:
                            nc.scalar.copy(
                                osr[:, jh * 512:(jh + 1) * 512], po[:])
                        else:
                            nc.vector.tensor_copy(
                                osr[:, jh * 512:(jh + 1) * 512], po[:])
                        if act_copy:
                            # tail: stream each half out as soon as its
                            # copy lands instead of waiting for the pair
                            nc.sync.dma_start(
                                out=out.ap()[g * 128:(g + 1) * 128,
                                             jh * 512:(jh + 1) * 512],
                                in_=osr[:, jh * 512:(jh + 1) * 512])
                        yield
                    if not act_copy:
                        nc.sync.dma_start(
                            out=out.ap()[g * 128:(g + 1) * 128, :],
                            in_=osr[:])

            def emit_oproj_qblock(qj, tail=False):
                drain(gen_oproj(qj, act_copy=tail))

            # keep-warm: tiny write-only matmuls fill PE idle so following
            # matmuls run at full clock instead of re-ramping.
            def emit_keepwarm(n, tag0):
                for i in range(n):
                    kwt = psA.tile([128, 512], f32, tag="proj",
                                   name=f"kw{tag0}{i}")
                    nc.tensor.matmul(kwt[:, 0:256], woT_sb[:, 0, 0:128],
                                     woT_sb[:, 0, 0:256],
                                     start=True, stop=True)

            # ---- schedule ----
            # Startup: q/k e-tile 0 of chunk 0 projects first so qj0 pair A
            # attention starts as early as possible; the rest of chunk 0
            # fills its PE gaps. Then qj order [0B, 1, 2, 3] with the next
            # projection chunk and the previous q-block's oproj interleaved
            # into each attention loop as PE filler. The tail is qj3's last
            # norms + its oproj, streamed out per half-tile.
            emit_vz()
            stq0 = stage_tensor("q", 0, split=True)
            nc.sync.dma_start(out=w3[:, 0, 0:2], in_=w3_r[:, 0, 0:2])
            nc.sync.dma_start(out=w3[:, 0, 2:NDT], in_=w3_r[:, 0, 2:NDT])
            nc.scalar.dma_start(out=bq_sb[:], in_=bqd.ap()[:])
            nc.scalar.dma_start(out=msk[:], in_=maskd.ap()[:])
            drain(gen_qk_ep("q", 0, 0, stq0))
            stk0 = stage_tensor("k", 0)
            nc.sync.dma_start(out=w3[:, 1], in_=w3_r[:, 1])
            drain(gen_qk_ep("k", 0, 0, stk0))
            stv0 = stage_tensor("v", 0)
            nc.sync.dma_start(out=w3[:, 2], in_=w3_r[:, 2])
            nc.scalar.dma_start(out=woT_sb[:], in_=woT_r[:])
            g0rest = chain(gen_v(0, stv0),
                           gen_qk_ep("q", 0, 1, stq0),
                           gen_qk_ep("k", 0, 1, stk0))
            emit_attention(0, pairs=(0,), defer_oproj=True, filler=g0rest)
            drain(g0rest)
            g1 = gen_chunk3(1)
            emit_attention(0, pairs=(1,), defer_oproj=True, filler=g1)
            drain(g1)
            g2 = chain(gen_chunk3(2), gen_oproj(0))
            emit_attention(1, defer_oproj=True, filler=g2)
            drain(g2)
            g3 = chain(gen_chunk3(3), gen_oproj(1))
            emit_attention(2, defer_oproj=True, filler=g3)
            drain(g3)
            emit_attention(3, defer_oproj=True, tail=True, kw=True,
                           filler=gen_oproj(2))
            emit_oproj_qblock(3, tail=True)

    nc.compile()
    return nc


_CACHE = {}


def kernel(q, k, v, mask, wq, bq, wk, bk, wv, bv, wo, bo):
    import ml_dtypes
    from concourse.bass_utils import run_bass_kernel_spmd
    npf8 = ml_dtypes.float8_e4m3

    q = np.asarray(q, np.float32)
    k = np.asarray(k, np.float32)
    v = np.asarray(v, np.float32)
    wq = np.asarray(wq, np.float32)
    wk = np.asarray(wk, np.float32)
    wv = np.asarray(wv, np.float32)
    wo = np.asarray(wo, np.float32)
    bq = np.asarray(bq, np.float32)
    bv = np.asarray(bv, np.float32)
    bo = np.asarray(bo, np.float32)

    cls, rng, pr, pats = _classify_mask(mask)
    pat_widths = [p.shape[1] for p in pats]
    key = (tuple(tuple(r) for r in cls), tuple(tuple(r) for r in rng),
           tuple(tuple(r) for r in pr), tuple(pat_widths))
    if key not in _CACHE:
        _CACHE[key] = _build_program(cls, rng, pr, pat_widths)
    nc = _CACHE[key]

    if pats:
        masks_np = np.ascontiguousarray(
            np.concatenate(pats, axis=1).astype(np.float16))
    else:
        masks_np = np.zeros((128, 1), np.float16)

    # per-batch transposed fp8 inputs
    qT8 = [np.ascontiguousarray(q[b].T).astype(npf8) for b in range(B)]
    kT8 = [np.ascontiguousarray(k[b].T).astype(npf8) for b in range(B)]
    vT8 = [np.ascontiguousarray(v[b].T).astype(npf8) for b in range(B)]

    def pack_w3(hg):
        el = slice(hg * EL, (hg + 1) * EL)
        ws = []
        for w, sc in ((wq, 16.0), (wk, 16.0), (wv, 1.0)):
            wt = np.ascontiguousarray(w[el, :].T * sc)    # [D, EL]
            ws.append(wt.reshape(NDT, 128, EL).transpose(1, 0, 2))
        return np.ascontiguousarray(
            np.stack(ws, axis=1).reshape(128, 3 * NDT * EL)).astype(npf8)

    in_maps = []
    for c in range(NC):
        b, hg = c // 4, c % 4
        el = slice(hg * EL, (hg + 1) * EL)
        woTl = np.ascontiguousarray(wo[:, el].T.astype(np.float16))
        m = {
            "qT": qT8[b], "kT": kT8[b], "vT": vT8[b],
            "w3": pack_w3(hg),
            "woT": np.ascontiguousarray(
                woTl.reshape(2, 128, D).transpose(1, 0, 2)
                .reshape(128, 2 * D)),
            "bq": np.ascontiguousarray(
                (bq[el] * 16.0).reshape(2, 128).T),
            "masks": masks_np,
        }
        in_maps.append(m)

    res = run_bass_kernel_spmd(nc, in_maps, list(range(NC)))
    accs = []
    for b in range(B):
        acc = res.results[b * 4]["out"].astype(np.float32)
        for hg in range(1, 4):
            acc = acc + res.results[b * 4 + hg]["out"]
        accs.append(acc)
    outf = np.stack(accs).reshape(B, S, D)
    # bo plus the folded V bias: softmax weights sum to 1 so the V bias
    # contributes wo @ bv to every output row
    outf = outf + (bo + wo @ bv)[None, None, :]
    return outf


# revision 20
# speedup vs baseline: 1.0191x; 1.0191x over previous
# Trainium2 Bass kernel for nn_MultiHeadAttention (B=2, S=2048, D=1024, H=16).
#
# Sharding: batch+head tensor-parallel over 8 cores. Core c handles batch
# c//4 and head-group c%4 (4 heads, 256 e-dims): column-sharded wq/wk/wv,
# row-sharded wo with the partial-output sum done on the host. Each core
# only reads its batch's q/k/v (12MB fp16 -> 6MB fp8 per core) and writes
# a [2048, 1024] fp16 partial.
#
# Projections run as fp8e4 DoubleRow matmuls (2 contraction tiles of 128
# per pass, 0.5 cyc/row): q/k/v stream fp8 from the host, wq/wk (and bq)
# pre-scaled by 16 so the fp8 weights stay clear of subnormals; the 16*16
# score scaling is folded into the softmax exp scale (0.125/256).
#
# Attention stays fp16 (DoubleRow dst must start at partition 0, which
# makes the Z ones-row trick impossible in DR mode):
#   QT/KT = (e 128, 2 e-tiles, 2048) computed via DR with weight tiles
#   stationary; V in natural (token, e) layout packed [V_h | ones] per
#   (kt, head) so AV yields row sums (Z) free in psum row 64.
#   ScoresT = (k, q) per head; exp PSUM->SBUF on ACT gives P^T directly.
#   Causal-boundary blocks multiply P^T with a resident 0/1 triangle tile
#   on DVE (fp16 2x mode).
# Bias handling: K bias dropped (softmax per-query shift invariance),
#   V bias folded into the host-side output bias (out += wo @ bv),
#   Q bias added on DVE during the PSUM->SBUF copy.
# Host sums the 4 partial outputs per batch in fp32 and adds bo + wo@bv.

import numpy as np

B, S, D, H = 2, 2048, 1024, 16
DK = D // H            # 64
NC = 8                 # cores
NH = 4                 # heads per core
EL = NH * DK           # 256 local e-dims
NCH = 4                # projection token-chunks per core
CH = S // NCH          # 512
NDT = D // 128         # 8 contraction tiles
NDP = NDT // 2         # 4 DoubleRow contraction pairs
NKT = S // 128         # 16 k-tiles
NQB = S // 512         # 4 q-blocks

SKIP, PLAIN = -1, -2   # block classes (>=0 means partial-pattern index)


def _classify_mask(mask):
    """Per (kt, qj) block classification of the (S_q, S_k) mask.

    Returns cls[kt][qj] (SKIP / PLAIN / pattern idx), rng[kt][qj] live col
    range, pr[kt][qj] partial col range, and the deduped 0/1 patterns
    (list of [128, w] float16 arrays) for the partial ranges."""
    m = np.asarray(mask).reshape(S, S)              # [q, k]; 0 = masked
    liveT = (m != 0).T                              # [k, q]
    cls = [[PLAIN] * NQB for _ in range(NKT)]
    rng = [[(0, 512)] * NQB for _ in range(NKT)]
    pr = [[(0, 0)] * NQB for _ in range(NKT)]
    uniq = {}
    pats = []
    for kt in range(NKT):
        for qj in range(NQB):
            blk = liveT[kt * 128:(kt + 1) * 128, qj * 512:(qj + 1) * 512]
            if blk.all():
                cls[kt][qj] = PLAIN
            elif not blk.any():
                cls[kt][qj] = SKIP
            else:
                live_col = blk.any(axis=0)
                nz = np.nonzero(live_col)[0]
                c0, c1 = int(nz[0]), int(nz[-1]) + 1
                rng[kt][qj] = (c0, c1)
                part_col = live_col & ~blk.all(axis=0)
                pz = np.nonzero(part_col)[0]
                p0, p1 = int(pz[0]), int(pz[-1]) + 1
                pr[kt][qj] = (p0, p1)
                pat = blk[:, p0:p1].astype(np.float16)
                key = (p1 - p0, pat.tobytes())
                if key not in uniq:
                    uniq[key] = len(pats)
                    pats.append(np.ascontiguousarray(pat))
                cls[kt][qj] = uniq[key]
    return cls, rng, pr, pats


def _build_program(cls, rng, pr, pat_widths):
    import concourse.bacc as bacc
    import concourse.mybir as mybir
    from concourse.tile import TileContext

    f32 = mybir.dt.float32
    f16 = mybir.dt.float16
    f8 = mybir.dt.float8e4
    DR = mybir.MatmulPerfMode.DoubleRow
    Exp = mybir.ActivationFunctionType.Exp
    mult = mybir.AluOpType.mult
    ESC = 0.125 / 256.0    # exp scale: 1/sqrt(DK) with the 16*16 w-scale

    # pattern offsets inside the resident mask tile
    moff = []
    o = 0
    for w in pat_widths:
        moff.append(o)
        o += w
    MW = max(o, 1)

    nc = bacc.Bacc("TRN2", target_bir_lowering=False, debug=False,
                   num_devices=NC)

    qT = nc.dram_tensor("qT", [D, S], f8, kind="ExternalInput")
    kT = nc.dram_tensor("kT", [D, S], f8, kind="ExternalInput")
    vT = nc.dram_tensor("vT", [D, S], f8, kind="ExternalInput")
    w3d = nc.dram_tensor("w3", [128, 3 * NDT * EL], f8,
                         kind="ExternalInput")
    woT = nc.dram_tensor("woT", [128, 2 * D], f16, kind="ExternalInput")
    bqd = nc.dram_tensor("bq", [128, 2], f32, kind="ExternalInput")
    maskd = nc.dram_tensor("masks", [128, MW], f16, kind="ExternalInput")
    out = nc.dram_tensor("out", [S, D], f16, kind="ExternalOutput")

    # transposed-input views: [p, t, c] with t the 128-row d-block
    qT_r = qT.ap().rearrange("(t p) c -> p t c", p=128)
    kT_r = kT.ap().rearrange("(t p) c -> p t c", p=128)
    vT_r = vT.ap().rearrange("(t p) c -> p t c", p=128)
    w3_r = w3d.ap().rearrange("p (j t e) -> p j t e", j=3, t=NDT)
    woT_r = woT.ap().rearrange("p (i e) -> p i e", i=2)

    with TileContext(nc) as tc:
        with (
            tc.tile_pool(name="const", bufs=1) as constp,
            tc.tile_pool(name="per", bufs=1) as perp,
            tc.tile_pool(name="stage", bufs=6) as stagep,
            tc.tile_pool(name="pt", bufs=10) as ptp,
            tc.tile_pool(name="zz", bufs=4) as zzp,
            tc.tile_pool(name="zb", bufs=8) as zbp,
            tc.tile_pool(name="ost", bufs=6) as ostp,
            tc.tile_pool(name="psA", bufs=2, space="PSUM") as psA,
            tc.tile_pool(name="psS", bufs=2, space="PSUM") as psS,
            tc.tile_pool(name="psO", bufs=2, space="PSUM") as psO,
        ):
            # ---- constants. w3 arrives per-tensor, interleaved with the
            # input staging DMAs, so the first matmuls start early ----
            w3 = constp.tile([128, 3, NDT, EL], f8, tag="w3")
            bq_sb = constp.tile([128, 2], f32, tag="bq")
            msk = constp.tile([128, MW], f16, tag="msk")
            woT_sb = constp.tile([128, 2, D], f16, tag="wo")

            # ---- persistent activations ----
            # QT/KT/OT: [p, e-tile, token]; head h -> (rows (h%2)*64,
            # e-tile h//2)
            QT_sb = perp.tile([128, 2, S], f16, tag="QT")
            KT_sb = perp.tile([128, 2, S], f16, tag="KT")
            OT_sb = perp.tile([128, 2, S], f16, tag="OT")
            # V natural (token, e) packed per (kt, head) as
            # [ones(64, last col=1) | V_h(64)] so the AV stationary
            # [ones-col | V] is a contiguous 65-col AP and Z (the exp row
            # sum) lands in psum row 0 -- partition 0, where the DVE
            # reciprocal can read it without a staging copy.
            V_big = perp.tile([128, NKT * NH * 2 * 64], f16, tag="Vb")
            V3 = V_big[:].rearrange("p (t x) -> p t x", x=64)

            def emit_vz():
                nc.vector.memset(V3[:, 0::2, :], 0.0)
                nc.vector.memset(V3[:, 0::2, 63:64], 1.0)

            # ---- projections (fp8 DoubleRow, 2 d-tiles per pass) ----
            # generators yield after each PE quantum (~0.2-0.4us) so the
            # attention loop can drain them as PE filler between groups
            JSRC = {"q": (0, qT_r), "k": (1, kT_r), "v": (2, vT_r)}

            def stage_tensor(name, c, split=False):
                j, src_r = JSRC[name]
                lo = c * CH
                st = stagep.tile([128, NDT, CH], f8, tag="stage")
                if split:
                    # land the first two d-tiles early so the PE starts
                    # before the full chunk arrives
                    nc.sync.dma_start(out=st[:, 0:2, :],
                                      in_=src_r[:, 0:2, lo:lo + CH])
                    nc.sync.dma_start(out=st[:, 2:NDT, :],
                                      in_=src_r[:, 2:NDT, lo:lo + CH])
                else:
                    nc.sync.dma_start(out=st[:], in_=src_r[:, :, lo:lo + CH])
                return st

            def gen_qk_ep(name, c, ep, st):
                # transposed layout [e, token]; out [64 e, 512] per piece;
                # psum [128, 512] holds 2 pieces (bases 0, 64)
                w = w3[:, JSRC[name][0]]
                lo = c * CH
                ps = psA.tile([128, 512], f32, tag="proj",
                              name=f"{name}p{c}{ep}")
                for pp in range(2):
                    sub = ps[pp * 64:pp * 64 + 64, :]
                    e0 = ep * 128 + pp * 64
                    for t in range(NDP):
                        nc.tensor.matmul(
                            sub, w[:, 2 * t:2 * t + 2, e0:e0 + 64],
                            st[:, 2 * t:2 * t + 2, :],
                            start=(t == 0), stop=(t == NDP - 1),
                            perf_mode=DR)
                    yield
                if name == "q":
                    nc.vector.tensor_scalar_add(
                        QT_sb[:, ep, lo:lo + CH], ps[:], bq_sb[:, ep:ep + 1])
                else:
                    nc.vector.tensor_copy(KT_sb[:, ep, lo:lo + CH], ps[:])

            def gen_v(c, st):
                # natural layout: out [64 tok, 256 e] per subtile; a psum
                # [128, 512] holds one kt (128 tokens) per col-half
                w = w3[:, 2]
                for half in range(2):
                    ps = psA.tile([128, 512], f32, tag="proj",
                                  name=f"vp{c}{half}")
                    for pos in range(4):       # (base, col-half)
                        t0 = half * 256 + pos * 64
                        ob = (pos % 2) * 64
                        oc = (pos // 2) * 256
                        sub = ps[ob:ob + 64, oc:oc + 256]
                        for t in range(NDP):
                            nc.tensor.matmul(
                                sub, st[:, 2 * t:2 * t + 2, t0:t0 + 64],
                                w[:, 2 * t:2 * t + 2, :],
                                start=(t == 0), stop=(t == NDP - 1),
                                perf_mode=DR)
                        if pos == 1:
                            yield
                    for kk in range(2):        # kt within this psum tile
                        kt = c * 4 + half * 2 + kk
                        nc.vector.tensor_copy(
                            V3[:, kt * NH * 2 + 1:(kt + 1) * NH * 2:2, :],
                            ps[:, kk * 256:(kk + 1) * 256]
                            .rearrange("p (h e) -> p h e", h=NH))
                    yield

            def gen_chunk3(c):
                stq = stage_tensor("q", c)
                yield from gen_qk_ep("q", c, 0, stq)
                yield from gen_qk_ep("q", c, 1, stq)
                stk = stage_tensor("k", c)
                yield from gen_qk_ep("k", c, 0, stk)
                yield from gen_qk_ep("k", c, 1, stk)
                stv = stage_tensor("v", c)
                yield from gen_v(c, stv)

            def chain(*gens):
                for g in gens:
                    yield from g

            def drain(g):
                for _ in g:
                    pass

            # ---- attention ----
            # Per q-block, heads processed as two interleaved pairs so PE
            # always has the other head's matmuls while ACT runs exp. The
            # AV matmul lags scores by one group so it never waits on exp,
            # and a caller-supplied filler (projection / oproj pieces) is
            # drained between groups to keep the PE queue dense.
            def emit_attention(qj, pairs=(0, 1), defer_oproj=False,
                               tail=False, kw=False, filler=None):
                qlo = qj * 512
                acts = [kt for kt in range(NKT) if cls[kt][qj] != SKIP]
                if not acts:
                    return
                groups = [acts[gi:gi + 2] for gi in range(0, len(acts), 2)]

                def pull_filler(n):
                    for _ in range(n):
                        if filler is not None:
                            try:
                                next(filler)
                                continue
                            except StopIteration:
                                pass
                        if kw:
                            emit_keepwarm(1, 700 + qj * 100
                                          + pull_filler.kwn)
                            pull_filler.kwn += 1
                        break
                pull_filler.kwn = 0

                def emit_scores(grp, h, sc):
                    ep, hp = h // 2, (h % 2) * 64
                    for i, kt in enumerate(grp):
                        c0, c1 = rng[kt][qj]
                        klo = kt * 128
                        nc.tensor.matmul(
                            sc[:, i * 512 + c0:i * 512 + c1],
                            KT_sb[hp:hp + 64, ep, klo:klo + 128],
                            QT_sb[hp:hp + 64, ep, qlo + c0:qlo + c1],
                            start=True, stop=True)

                def emit_exp_mask(grp, pt, sc):
                    spans = [(i * 512 + rng[kt][qj][0],
                              i * 512 + rng[kt][qj][1])
                             for i, kt in enumerate(grp)]
                    lo, hi2 = spans[0][0], spans[-1][1]
                    dead = (hi2 - lo) - sum(b - a for a, b in spans)
                    exp_spans = spans if dead > 200 else [(lo, hi2)]
                    for a, bnd in exp_spans:
                        nc.scalar.activation(pt[:, a:bnd], sc[:, a:bnd],
                                             Exp, scale=ESC)
                    for i, kt in enumerate(grp):
                        cl = cls[kt][qj]
                        if cl >= 0:
                            pp0, pp1 = pr[kt][qj]
                            sl = slice(i * 512 + pp0, i * 512 + pp1)
                            nc.vector.tensor_tensor(
                                pt[:, sl], pt[:, sl],
                                msk[:, moff[cl]:moff[cl] + pp1 - pp0],
                                op=mult)

                def emit_av(grp, hi, h, pt, ots, n_done):
                    for i, kt in enumerate(grp):
                        c0, c1 = rng[kt][qj]
                        base = (kt * NH + h) * 128
                        vap = V_big[:, base + 63:base + 128]
                        n_done[hi] += 1
                        nc.tensor.matmul(
                            ots[hi][0:65, c0:c1], vap,
                            pt[:, i * 512 + c0:i * 512 + c1],
                            start=(n_done[hi] == 1),
                            stop=(n_done[hi] == len(acts)))

                for pair in pairs:
                    hs = (pair * 2, pair * 2 + 1)   # local head ids
                    ots = [psO.tile([128, 512], f32, tag="ot",
                                    name=f"ot{qj}{h}") for h in hs]
                    n_done = [0, 0]
                    pend = []   # (grp, hi, h, pt) AV lagging one group
                    for gidx, grp in enumerate(groups):
                        for hi, h in enumerate(hs):
                            sc = psS.tile([128, 1024], f32, tag="score")
                            emit_scores(grp, h, sc)
                            pt = ptp.tile([128, 1024], f16, tag="pt")
                            emit_exp_mask(grp, pt, sc)
                            if pend:
                                emit_av(*pend.pop(0), ots, n_done)
                            pend.append((grp, hi, h, pt))
                            pull_filler(1)
                    while pend:
                        emit_av(*pend.pop(0), ots, n_done)
                    for hi, h in enumerate(hs):
                        # normalize: row 0 of ot = Z (the ones column).
                        # 65-row copy moves Z+AV to SBUF and frees the
                        # PSUM tile early; the reciprocal reads Z at
                        # partition 0 with no staging copy.
                        ep, hp = h // 2, (h % 2) * 64
                        ot = ots[hi]
                        oc = zbp.tile([65, 512], f32, tag="oc")
                        if tail and pair == pairs[-1]:
                            # ACT is idle once the last exps are done
                            nc.scalar.copy(oc[:], ot[0:65, :])
                        else:
                            nc.vector.tensor_copy(oc[:], ot[0:65, :])
                        rz = zzp.tile([1, 512], f32, tag="z")
                        nc.vector.reciprocal_approx_fast(rz[:], oc[0:1, :])
                        rb = zbp.tile([64, 512], f32, tag="zb")
                        nc.gpsimd.partition_broadcast(rb[:], rz[:],
                                                      channels=64)
                        nc.vector.tensor_tensor(
                            OT_sb[hp:hp + 64, ep, qlo:qlo + 512],
                            oc[1:65, :], rb[:], op=mult)
                if not defer_oproj:
                    emit_oproj_qblock(qj)

            # ---- output projection (partial over local e-dims) ----
            # generator yielding per half-tile; psA ring (transient, like
            # the projection pieces it interleaves with)
            def gen_oproj(qj, act_copy=False):
                for g in range(qj * 4, (qj + 1) * 4):
                    osr = ostp.tile([128, D], f16, tag="ost")
                    for jh in range(2):
                        po = psA.tile([128, 512], f32, tag="proj",
                                      name=f"po{g}{jh}")
                        for ep in range(2):
                            nc.tensor.matmul(
                                po[:], OT_sb[:, ep, g * 128:(g + 1) * 128],
                                woT_sb[:, ep, jh * 512:(jh + 1) * 512],
                                start=(ep == 0), stop=(ep == 1))
                        if act_copy and jh == 0:
                            nc.scalar.copy(
                                osr[:, jh * 512:(jh + 1) * 512], po[:])
                        else:
                            nc.vector.tensor_copy(
                                osr[:, jh * 512:(jh + 1) * 512], po[:])
                        if act_copy:
                            # tail: stream each half out as soon as its
                            # copy lands instead of waiting for the pair
                            nc.sync.dma_start(
                                out=out.ap()[g * 128:(g + 1) * 128,
                                             jh * 512:(jh + 1) * 512],
                                in_=osr[:, jh * 512:(jh + 1) * 512])
                        yield
                    if not act_copy:
                        nc.sync.dma_start(
                            out=out.ap()[g * 128:(g + 1) * 128, :],
                            in_=osr[:])

            def emit_oproj_qblock(qj, tail=False):
                drain(gen_oproj(qj, act_copy=tail))

            # keep-warm: tiny write-only matmuls fill PE idle so following
            # matmuls run at full clock instead of re-ramping.
            def emit_keepwarm(n, tag0):
                for i in range(n):
                    kwt = psA.tile([128, 512], f32, tag="proj",
                                   name=f"kw{tag0}{i}")
                    nc.tensor.matmul(kwt[:, 0:256], woT_sb[:, 0, 0:128],
                                     woT_sb[:, 0, 0:256],
                                     start=True, stop=True)

            # ---- schedule ----
            # Startup: q/k e-tile 0 of chunk 0 projects first so qj0 pair A
            # attention starts as early as possible; the rest of chunk 0
            # fills its PE gaps. Then qj order [0B, 1, 2, 3] with the next
            # projection chunk and the previous q-block's oproj interleaved
            # into each attention loop as PE filler. The tail is qj3's last
            # norms + its oproj, streamed out per half-tile.
            emit_vz()
            stq0 = stage_tensor("q", 0, split=True)
            nc.sync.dma_start(out=w3[:, 0, 0:2], in_=w3_r[:, 0, 0:2])
            nc.sync.dma_start(out=w3[:, 0, 2:NDT], in_=w3_r[:, 0, 2:NDT])
            nc.scalar.dma_start(out=bq_sb[:], in_=bqd.ap()[:])
            nc.scalar.dma_start(out=msk[:], in_=maskd.ap()[:])
            drain(gen_qk_ep("q", 0, 0, stq0))
            stk0 = stage_tensor("k", 0)
            nc.sync.dma_start(out=w3[:, 1], in_=w3_r[:, 1])
            drain(gen_qk_ep("k", 0, 0, stk0))
            stv0 = stage_tensor("v", 0)
            nc.sync.dma_start(out=w3[:, 2], in_=w3_r[:, 2])
            nc.scalar.dma_start(out=woT_sb[:], in_=woT_r[:])
            g0rest = chain(gen_v(0, stv0),
                           gen_qk_ep("q", 0, 1, stq0),
                           gen_qk_ep("k", 0, 1, stk0))
            emit_attention(0, pairs=(0,), defer_oproj=True, filler=g0rest)
            drain(g0rest)
            g1 = gen_chunk3(1)
            emit_attention(0, pairs=(1,), defer_oproj=True, filler=g1)
            drain(g1)
            g2 = chain(gen_chunk3(2), gen_oproj(0))
            emit_attention(1, defer_oproj=True, filler=g2)
            drain(g2)
            g3 = chain(gen_chunk3(3), gen_oproj(1))
            emit_attention(2, defer_oproj=True, filler=g3)
            drain(g3)
            emit_attention(3, defer_oproj=True, tail=True, kw=True,
                           filler=gen_oproj(2))
            emit_oproj_qblock(3, tail=True)

    nc.compile()
    return nc


_CACHE = {}


def kernel(q, k, v, mask, wq, bq, wk, bk, wv, bv, wo, bo):
    import ml_dtypes
    from concourse.bass_utils import run_bass_kernel_spmd
    npf8 = ml_dtypes.float8_e4m3

    q = np.asarray(q, np.float32)
    k = np.asarray(k, np.float32)
    v = np.asarray(v, np.float32)
    wq = np.asarray(wq, np.float32)
    wk = np.asarray(wk, np.float32)
    wv = np.asarray(wv, np.float32)
    wo = np.asarray(wo, np.float32)
    bq = np.asarray(bq, np.float32)
    bv = np.asarray(bv, np.float32)
    bo = np.asarray(bo, np.float32)

    cls, rng, pr, pats = _classify_mask(mask)
    pat_widths = [p.shape[1] for p in pats]
    key = (tuple(tuple(r) for r in cls), tuple(tuple(r) for r in rng),
           tuple(tuple(r) for r in pr), tuple(pat_widths))
    if key not in _CACHE:
        _CACHE[key] = _build_program(cls, rng, pr, pat_widths)
    nc = _CACHE[key]

    if pats:
        masks_np = np.ascontiguousarray(
            np.concatenate(pats, axis=1).astype(np.float16))
    else:
        masks_np = np.zeros((128, 1), np.float16)

    # per-batch transposed fp8 inputs
    qT8 = [np.ascontiguousarray(q[b].T).astype(npf8) for b in range(B)]
    kT8 = [np.ascontiguousarray(k[b].T).astype(npf8) for b in range(B)]
    vT8 = [np.ascontiguousarray(v[b].T).astype(npf8) for b in range(B)]

    def pack_w3(hg):
        el = slice(hg * EL, (hg + 1) * EL)
        ws = []
        for w, sc in ((wq, 16.0), (wk, 16.0), (wv, 1.0)):
            wt = np.ascontiguousarray(w[el, :].T * sc)    # [D, EL]
            ws.append(wt.reshape(NDT, 128, EL).transpose(1, 0, 2))
        return np.ascontiguousarray(
            np.stack(ws, axis=1).reshape(128, 3 * NDT * EL)).astype(npf8)

    in_maps = []
    for c in range(NC):
        b, hg = c // 4, c % 4
        el = slice(hg * EL, (hg + 1) * EL)
        woTl = np.ascontiguousarray(wo[:, el].T.astype(np.float16))
        m = {
            "qT": qT8[b], "kT": kT8[b], "vT": vT8[b],
            "w3": pack_w3(hg),
            "woT": np.ascontiguousarray(
                woTl.reshape(2, 128, D).transpose(1, 0, 2)
                .reshape(128, 2 * D)),
            "bq": np.ascontiguousarray(
                (bq[el] * 16.0).reshape(2, 128).T),
            "masks": masks_np,
        }
        in_maps.append(m)

    res = run_bass_kernel_spmd(nc, in_maps, list(range(NC)))
    accs = []
    for b in range(B):
        acc = res.results[b * 4]["out"].astype(np.float32)
        for hg in range(1, 4):
            acc = acc + res.results[b * 4 + hg]["out"]
        accs.append(acc)
    outf = np.stack(accs).reshape(B, S, D)
    # bo plus the folded V bias: softmax weights sum to 1 so the V bias
    # contributes wo @ bv to every output row
    outf = outf + (bo + wo @ bv)[None, None, :]
    return outf


# revision 24
# speedup vs baseline: 1.0247x; 1.0055x over previous
# Trainium2 Bass kernel for nn_MultiHeadAttention (B=2, S=2048, D=1024, H=16).
#
# Sharding: batch+head tensor-parallel over 8 cores. Core c handles batch
# c//4 and head-group c%4 (4 heads, 256 e-dims): column-sharded wq/wk/wv,
# row-sharded wo with the partial-output sum done on the host. Each core
# only reads its batch's q/k/v (12MB fp16 -> 6MB fp8 per core) and writes
# a [2048, 1024] fp16 partial.
#
# Projections run as fp8e4 DoubleRow matmuls (2 contraction tiles of 128
# per pass, 0.5 cyc/row): q/k/v stream fp8 from the host, wq/wk (and bq)
# pre-scaled by 16 so the fp8 weights stay clear of subnormals; the 16*16
# score scaling is folded into the softmax exp scale (0.125/256).
#
# Attention stays fp16 (DoubleRow dst must start at partition 0, which
# makes the Z ones-row trick impossible in DR mode):
#   QT/KT = (e 128, 2 e-tiles, 2048) computed via DR with weight tiles
#   stationary; V in natural (token, e) layout packed [V_h | ones] per
#   (kt, head) so AV yields row sums (Z) free in psum row 64.
#   ScoresT = (k, q) per head; exp PSUM->SBUF on ACT gives P^T directly.
#   Causal-boundary blocks multiply P^T with a resident 0/1 triangle tile
#   on DVE (fp16 2x mode).
# Bias handling: K bias dropped (softmax per-query shift invariance),
#   V bias folded into the host-side output bias (out += wo @ bv),
#   Q bias added on DVE during the PSUM->SBUF copy.
# Host sums the 4 partial outputs per batch in fp32 and adds bo + wo@bv.

import numpy as np

B, S, D, H = 2, 2048, 1024, 16
DK = D // H            # 64
NC = 8                 # cores
NH = 4                 # heads per core
EL = NH * DK           # 256 local e-dims
NCH = 4                # projection token-chunks per core
CH = S // NCH          # 512
NDT = D // 128         # 8 contraction tiles
NDP = NDT // 2         # 4 DoubleRow contraction pairs
NKT = S // 128         # 16 k-tiles
NQB = S // 512         # 4 q-blocks

SKIP, PLAIN = -1, -2   # block classes (>=0 means partial-pattern index)


def _classify_mask(mask):
    """Per (kt, qj) block classification of the (S_q, S_k) mask.

    Returns cls[kt][qj] (SKIP / PLAIN / pattern idx), rng[kt][qj] live col
    range, pr[kt][qj] partial col range, and the deduped 0/1 patterns
    (list of [128, w] float16 arrays) for the partial ranges."""
    m = np.asarray(mask).reshape(S, S)              # [q, k]; 0 = masked
    liveT = (m != 0).T                              # [k, q]
    cls = [[PLAIN] * NQB for _ in range(NKT)]
    rng = [[(0, 512)] * NQB for _ in range(NKT)]
    pr = [[(0, 0)] * NQB for _ in range(NKT)]
    uniq = {}
    pats = []
    for kt in range(NKT):
        for qj in range(NQB):
            blk = liveT[kt * 128:(kt + 1) * 128, qj * 512:(qj + 1) * 512]
            if blk.all():
                cls[kt][qj] = PLAIN
            elif not blk.any():
                cls[kt][qj] = SKIP
            else:
                live_col = blk.any(axis=0)
                nz = np.nonzero(live_col)[0]
                c0, c1 = int(nz[0]), int(nz[-1]) + 1
                rng[kt][qj] = (c0, c1)
                part_col = live_col & ~blk.all(axis=0)
                pz = np.nonzero(part_col)[0]
                p0, p1 = int(pz[0]), int(pz[-1]) + 1
                pr[kt][qj] = (p0, p1)
                pat = blk[:, p0:p1].astype(np.float16)
                key = (p1 - p0, pat.tobytes())
                if key not in uniq:
                    uniq[key] = len(pats)
                    pats.append(np.ascontiguousarray(pat))
                cls[kt][qj] = uniq[key]
    return cls, rng, pr, pats


def _build_program(cls, rng, pr, pat_widths):
    import concourse.bacc as bacc
    import concourse.mybir as mybir
    from concourse.tile import TileContext

    f32 = mybir.dt.float32
    f16 = mybir.dt.float16
    f8 = mybir.dt.float8e4
    DR = mybir.MatmulPerfMode.DoubleRow
    Exp = mybir.ActivationFunctionType.Exp
    mult = mybir.AluOpType.mult
    ESC = 0.125 / 256.0    # exp scale: 1/sqrt(DK) with the 16*16 w-scale

    # pattern offsets inside the resident mask tile
    moff = []
    o = 0
    for w in pat_widths:
        moff.append(o)
        o += w
    MW = max(o, 1)

    nc = bacc.Bacc("TRN2", target_bir_lowering=False, debug=False,
                   num_devices=NC)

    qT = nc.dram_tensor("qT", [D, S], f8, kind="ExternalInput")
    kT = nc.dram_tensor("kT", [D, S], f8, kind="ExternalInput")
    vT = nc.dram_tensor("vT", [D, S], f8, kind="ExternalInput")
    w3d = nc.dram_tensor("w3", [128, 3 * NDT * EL], f8,
                         kind="ExternalInput")
    woT = nc.dram_tensor("woT", [128, 2 * D], f16, kind="ExternalInput")
    bqd = nc.dram_tensor("bq", [128, 2], f32, kind="ExternalInput")
    maskd = nc.dram_tensor("masks", [128, MW], f16, kind="ExternalInput")
    out = nc.dram_tensor("out", [S, D], f16, kind="ExternalOutput")

    # transposed-input views: [p, t, c] with t the 128-row d-block
    qT_r = qT.ap().rearrange("(t p) c -> p t c", p=128)
    kT_r = kT.ap().rearrange("(t p) c -> p t c", p=128)
    vT_r = vT.ap().rearrange("(t p) c -> p t c", p=128)
    w3_r = w3d.ap().rearrange("p (j t e) -> p j t e", j=3, t=NDT)
    woT_r = woT.ap().rearrange("p (i e) -> p i e", i=2)

    with TileContext(nc) as tc:
        with (
            tc.tile_pool(name="const", bufs=1) as constp,
            tc.tile_pool(name="per", bufs=1) as perp,
            tc.tile_pool(name="stage", bufs=6) as stagep,
            tc.tile_pool(name="pt", bufs=10) as ptp,
            tc.tile_pool(name="zz", bufs=4) as zzp,
            tc.tile_pool(name="zb", bufs=8) as zbp,
            tc.tile_pool(name="ost", bufs=6) as ostp,
            tc.tile_pool(name="psA", bufs=2, space="PSUM") as psA,
            tc.tile_pool(name="psS", bufs=2, space="PSUM") as psS,
            tc.tile_pool(name="psO", bufs=2, space="PSUM") as psO,
        ):
            # ---- constants. w3 arrives per-tensor, interleaved with the
            # input staging DMAs, so the first matmuls start early ----
            w3 = constp.tile([128, 3, NDT, EL], f8, tag="w3")
            bq_sb = constp.tile([128, 2], f32, tag="bq")
            msk = constp.tile([128, MW], f16, tag="msk")
            woT_sb = constp.tile([128, 2, D], f16, tag="wo")

            # ---- persistent activations ----
            # QT/KT/OT: [p, e-tile, token]; head h -> (rows (h%2)*64,
            # e-tile h//2)
            QT_sb = perp.tile([128, 2, S], f16, tag="QT")
            KT_sb = perp.tile([128, 2, S], f16, tag="KT")
            OT_sb = perp.tile([128, 2, S], f16, tag="OT")
            # V natural (token, e) packed per (kt, head) as
            # [ones(64, last col=1) | V_h(64)] so the AV stationary
            # [ones-col | V] is a contiguous 65-col AP and Z (the exp row
            # sum) lands in psum row 0 -- partition 0, where the DVE
            # reciprocal can read it without a staging copy.
            V_big = perp.tile([128, NKT * NH * 2 * 64], f16, tag="Vb")
            V3 = V_big[:].rearrange("p (t x) -> p t x", x=64)

            def emit_vz():
                nc.vector.memset(V3[:, 0::2, :], 0.0)
                nc.vector.memset(V3[:, 0::2, 63:64], 1.0)

            # ---- projections (fp8 DoubleRow, 2 d-tiles per pass) ----
            # generators yield after each PE quantum (~0.2-0.4us) so the
            # attention loop can drain them as PE filler between groups
            JSRC = {"q": (0, qT_r), "k": (1, kT_r), "v": (2, vT_r)}

            def stage_tensor(name, c, split=False):
                j, src_r = JSRC[name]
                lo = c * CH
                st = stagep.tile([128, NDT, CH], f8, tag="stage")
                if split:
                    # land the first two d-tiles early so the PE starts
                    # before the full chunk arrives
                    nc.sync.dma_start(out=st[:, 0:2, :],
                                      in_=src_r[:, 0:2, lo:lo + CH])
                    nc.sync.dma_start(out=st[:, 2:NDT, :],
                                      in_=src_r[:, 2:NDT, lo:lo + CH])
                else:
                    nc.sync.dma_start(out=st[:], in_=src_r[:, :, lo:lo + CH])
                return st

            def gen_qk_ep(name, c, ep, st):
                # transposed layout [e, token]; out [64 e, 512] per piece;
                # psum [128, 512] holds 2 pieces (bases 0, 64)
                w = w3[:, JSRC[name][0]]
                lo = c * CH
                ps = psA.tile([128, 512], f32, tag="proj",
                              name=f"{name}p{c}{ep}")
                for pp in range(2):
                    sub = ps[pp * 64:pp * 64 + 64, :]
                    e0 = ep * 128 + pp * 64
                    for t in range(NDP):
                        nc.tensor.matmul(
                            sub, w[:, 2 * t:2 * t + 2, e0:e0 + 64],
                            st[:, 2 * t:2 * t + 2, :],
                            start=(t == 0), stop=(t == NDP - 1),
                            perf_mode=DR)
                    yield
                if name == "q":
                    nc.vector.tensor_scalar_add(
                        QT_sb[:, ep, lo:lo + CH], ps[:], bq_sb[:, ep:ep + 1])
                else:
                    nc.vector.tensor_copy(KT_sb[:, ep, lo:lo + CH], ps[:])

            def gen_v(c, st):
                # natural layout: out [64 tok, 256 e] per subtile; a psum
                # [128, 512] holds one kt (128 tokens) per col-half
                w = w3[:, 2]
                for half in range(2):
                    ps = psA.tile([128, 512], f32, tag="proj",
                                  name=f"vp{c}{half}")
                    for pos in range(4):       # (base, col-half)
                        t0 = half * 256 + pos * 64
                        ob = (pos % 2) * 64
                        oc = (pos // 2) * 256
                        sub = ps[ob:ob + 64, oc:oc + 256]
                        for t in range(NDP):
                            nc.tensor.matmul(
                                sub, st[:, 2 * t:2 * t + 2, t0:t0 + 64],
                                w[:, 2 * t:2 * t + 2, :],
                                start=(t == 0), stop=(t == NDP - 1),
                                perf_mode=DR)
                        if pos == 1:
                            yield
                    for kk in range(2):        # kt within this psum tile
                        kt = c * 4 + half * 2 + kk
                        nc.vector.tensor_copy(
                            V3[:, kt * NH * 2 + 1:(kt + 1) * NH * 2:2, :],
                            ps[:, kk * 256:(kk + 1) * 256]
                            .rearrange("p (h e) -> p h e", h=NH))
                    yield

            def gen_chunk3(c):
                # e-tile 0 of q AND k first: the next q-block's pair-A
                # scores need exactly those
                stq = stage_tensor("q", c)
                stk = stage_tensor("k", c)
                yield from gen_qk_ep("q", c, 0, stq)
                yield from gen_qk_ep("k", c, 0, stk)
                yield from gen_qk_ep("q", c, 1, stq)
                yield from gen_qk_ep("k", c, 1, stk)
                stv = stage_tensor("v", c)
                yield from gen_v(c, stv)

            def chain(*gens):
                for g in gens:
                    yield from g

            def drain(g):
                for _ in g:
                    pass

            # ---- attention ----
            # Per q-block, heads processed as two interleaved pairs so PE
            # always has the other head's matmuls while ACT runs exp. The
            # AV matmul lags scores by one group so it never waits on exp,
            # and a caller-supplied filler (projection / oproj pieces) is
            # drained between groups to keep the PE queue dense.
            def emit_attention(qj, pairs=(0, 1), defer_oproj=False,
                               tail=False, kw=False, filler=None, pulls=1):
                qlo = qj * 512
                acts = [kt for kt in range(NKT) if cls[kt][qj] != SKIP]
                if not acts:
                    return
                groups = [acts[gi:gi + 2] for gi in range(0, len(acts), 2)]

                def pull_filler(n):
                    for _ in range(n):
                        if filler is not None:
                            try:
                                next(filler)
                                continue
                            except StopIteration:
                                pass
                        if kw:
                            emit_keepwarm(1, 700 + qj * 100
                                          + pull_filler.kwn)
                            pull_filler.kwn += 1
                        break
                pull_filler.kwn = 0

                def emit_scores(grp, h, sc):
                    ep, hp = h // 2, (h % 2) * 64
                    for i, kt in enumerate(grp):
                        c0, c1 = rng[kt][qj]
                        klo = kt * 128
                        nc.tensor.matmul(
                            sc[:, i * 512 + c0:i * 512 + c1],
                            KT_sb[hp:hp + 64, ep, klo:klo + 128],
                            QT_sb[hp:hp + 64, ep, qlo + c0:qlo + c1],
                            start=True, stop=True)

                def emit_exp_mask(grp, pt, sc):
                    spans = [(i * 512 + rng[kt][qj][0],
                              i * 512 + rng[kt][qj][1])
                             for i, kt in enumerate(grp)]
                    lo, hi2 = spans[0][0], spans[-1][1]
                    dead = (hi2 - lo) - sum(b - a for a, b in spans)
                    exp_spans = spans if dead > 200 else [(lo, hi2)]
                    for a, bnd in exp_spans:
                        nc.scalar.activation(pt[:, a:bnd], sc[:, a:bnd],
                                             Exp, scale=ESC)
                    for i, kt in enumerate(grp):
                        cl = cls[kt][qj]
                        if cl >= 0:
                            pp0, pp1 = pr[kt][qj]
                            sl = slice(i * 512 + pp0, i * 512 + pp1)
                            nc.vector.tensor_tensor(
                                pt[:, sl], pt[:, sl],
                                msk[:, moff[cl]:moff[cl] + pp1 - pp0],
                                op=mult)

                def emit_av(grp, hi, h, pt, ots, n_done):
                    for i, kt in enumerate(grp):
                        c0, c1 = rng[kt][qj]
                        base = (kt * NH + h) * 128
                        vap = V_big[:, base + 63:base + 128]
                        n_done[hi] += 1
                        nc.tensor.matmul(
                            ots[hi][0:65, c0:c1], vap,
                            pt[:, i * 512 + c0:i * 512 + c1],
                            start=(n_done[hi] == 1),
                            stop=(n_done[hi] == len(acts)))

                def emit_norm(pair, hi, h, ots):
                    # normalize: row 0 of ot = Z (the ones column). 65-row
                    # copy moves Z+AV to SBUF and frees the PSUM tile
                    # early; the reciprocal reads Z at partition 0 with no
                    # staging copy.
                    ep, hp = h // 2, (h % 2) * 64
                    ot = ots[hi]
                    oc = zbp.tile([65, 512], f32, tag="oc")
                    if tail and pair == pairs[-1] and hi == 0:
                        # ACT is idle once the last exps are done
                        nc.scalar.copy(oc[:], ot[0:65, :])
                    else:
                        nc.vector.tensor_copy(oc[:], ot[0:65, :])
                    rz = zzp.tile([1, 512], f32, tag="z")
                    nc.vector.reciprocal_approx_fast(rz[:], oc[0:1, :])
                    rb = zbp.tile([64, 512], f32, tag="zb")
                    nc.gpsimd.partition_broadcast(rb[:], rz[:],
                                                  channels=64)
                    nc.vector.tensor_tensor(
                        OT_sb[hp:hp + 64, ep, qlo:qlo + 512],
                        oc[1:65, :], rb[:], op=mult)

                # Flat item stream; with two pairs, pair B's first group is
                # hoisted into pair A's last so the exp pipeline never
                # drains at the pair boundary.
                def pair_items(pair):
                    return [(pair, grp, hi) for grp in groups
                            for hi in range(2)]

                if len(pairs) == 2:
                    a, bb = pair_items(pairs[0]), pair_items(pairs[1])
                    stream = a[:-2]
                    for x, y in zip(bb[:2], a[-2:]):
                        stream += [x, y]
                    stream += bb[2:]
                else:
                    stream = pair_items(pairs[0])

                ots = {}
                n_done = {}
                n_av = {p: 0 for p in pairs}
                pend = []   # (pair, grp, hi, h, pt): AV lags one item

                def flush_av():
                    pair, grp, hi, h, pt = pend.pop(0)
                    emit_av(grp, hi, h, pt, ots[pair], n_done[pair])
                    n_av[pair] += len(grp)
                    if n_av[pair] == 2 * len(acts):
                        for hj in range(2):
                            emit_norm(pair, hj, pair * 2 + hj, ots[pair])

                for idx, (pair, grp, hi) in enumerate(stream):
                    if pair not in ots:
                        ots[pair] = [psO.tile([128, 512], f32, tag="ot",
                                              name=f"ot{qj}{pair * 2 + j}")
                                     for j in range(2)]
                        n_done[pair] = [0, 0]
                    h = pair * 2 + hi
                    sc = psS.tile([128, 1024], f32, tag="score")
                    emit_scores(grp, h, sc)
                    pt = ptp.tile([128, 1024], f16, tag="pt")
                    emit_exp_mask(grp, pt, sc)
                    if pend:
                        flush_av()
                    pend.append((pair, grp, hi, h, pt))
                    if kw and idx >= len(stream) - 6:
                        pass    # keep the critical tail free of filler
                    else:
                        pull_filler(pulls)
                while pend:
                    flush_av()
                if not defer_oproj:
                    emit_oproj_qblock(qj)

            # ---- output projection (partial over local e-dims) ----
            # generator yielding per half-tile; psA ring (transient, like
            # the projection pieces it interleaves with)
            def gen_oproj(qj, act_copy=False):
                for g in range(qj * 4, (qj + 1) * 4):
                    osr = ostp.tile([128, D], f16, tag="ost")
                    for jh in range(2):
                        po = psA.tile([128, 512], f32, tag="proj",
                                      name=f"po{g}{jh}")
                        for ep in range(2):
                            nc.tensor.matmul(
                                po[:], OT_sb[:, ep, g * 128:(g + 1) * 128],
                                woT_sb[:, ep, jh * 512:(jh + 1) * 512],
                                start=(ep == 0), stop=(ep == 1))
                        if act_copy and jh == 0:
                            nc.scalar.copy(
                                osr[:, jh * 512:(jh + 1) * 512], po[:])
                        else:
                            nc.vector.tensor_copy(
                                osr[:, jh * 512:(jh + 1) * 512], po[:])
                        if act_copy:
                            # tail: stream each half out as soon as its
                            # copy lands instead of waiting for the pair
                            nc.sync.dma_start(
                                out=out.ap()[g * 128:(g + 1) * 128,
                                             jh * 512:(jh + 1) * 512],
                                in_=osr[:, jh * 512:(jh + 1) * 512])
                        yield
                    if not act_copy:
                        nc.sync.dma_start(
                            out=out.ap()[g * 128:(g + 1) * 128, :],
                            in_=osr[:])

            def emit_oproj_qblock(qj, tail=False):
                drain(gen_oproj(qj, act_copy=tail))

            # keep-warm: tiny write-only matmuls fill PE idle so following
            # matmuls run at full clock instead of re-ramping.
            def emit_keepwarm(n, tag0):
                for i in range(n):
                    kwt = psA.tile([128, 512], f32, tag="proj",
                                   name=f"kw{tag0}{i}")
                    nc.tensor.matmul(kwt[:, 0:256], woT_sb[:, 0, 0:128],
                                     woT_sb[:, 0, 0:256],
                                     start=True, stop=True)

            # ---- schedule ----
            # Startup: q/k e-tile 0 of chunk 0 projects first so qj0 pair A
            # attention starts as early as possible; the rest of chunk 0
            # fills its PE gaps. Then qj order [0B, 1, 2, 3] with the next
            # projection chunk and the previous q-block's oproj interleaved
            # into each attention loop as PE filler. The tail is qj3's last
            # norms + its oproj, streamed out per half-tile.
            emit_vz()
            nc.sync.dma_start(out=w3[:, 0], in_=w3_r[:, 0])
            nc.scalar.dma_start(out=bq_sb[:], in_=bqd.ap()[:])
            nc.scalar.dma_start(out=msk[:], in_=maskd.ap()[:])
            stq0 = stage_tensor("q", 0, split=True)
            drain(gen_qk_ep("q", 0, 0, stq0))
            nc.sync.dma_start(out=w3[:, 1], in_=w3_r[:, 1])
            stk0 = stage_tensor("k", 0, split=True)
            drain(gen_qk_ep("k", 0, 0, stk0))
            nc.sync.dma_start(out=w3[:, 2], in_=w3_r[:, 2])
            stv0 = stage_tensor("v", 0)
            nc.scalar.dma_start(out=woT_sb[:], in_=woT_r[:])
            g0rest = chain(gen_v(0, stv0),
                           gen_qk_ep("q", 0, 1, stq0),
                           gen_qk_ep("k", 0, 1, stk0))
            emit_attention(0, pairs=(0,), defer_oproj=True, filler=g0rest)
            drain(g0rest)
            g1 = gen_chunk3(1)
            emit_attention(0, pairs=(1,), defer_oproj=True, filler=g1,
                           pulls=3)
            drain(g1)
            g2 = chain(gen_chunk3(2), gen_oproj(0))
            emit_attention(1, defer_oproj=True, filler=g2, pulls=2)
            drain(g2)
            g3 = chain(gen_chunk3(3), gen_oproj(1))
            emit_attention(2, defer_oproj=True, filler=g3)
            drain(g3)
            emit_attention(3, defer_oproj=True, tail=True, kw=True,
                           filler=gen_oproj(2))
            emit_oproj_qblock(3, tail=True)

    nc.compile()
    return nc


_CACHE = {}


def kernel(q, k, v, mask, wq, bq, wk, bk, wv, bv, wo, bo):
    import ml_dtypes
    from concourse.bass_utils import run_bass_kernel_spmd
    npf8 = ml_dtypes.float8_e4m3

    q = np.asarray(q, np.float32)
    k = np.asarray(k, np.float32)
    v = np.asarray(v, np.float32)
    wq = np.asarray(wq, np.float32)
    wk = np.asarray(wk, np.float32)
    wv = np.asarray(wv, np.float32)
    wo = np.asarray(wo, np.float32)
    bq = np.asarray(bq, np.float32)
    bv = np.asarray(bv, np.float32)
    bo = np.asarray(bo, np.float32)

    cls, rng, pr, pats = _classify_mask(mask)
    pat_widths = [p.shape[1] for p in pats]
    key = (tuple(tuple(r) for r in cls), tuple(tuple(r) for r in rng),
           tuple(tuple(r) for r in pr), tuple(pat_widths))
    if key not in _CACHE:
        _CACHE[key] = _build_program(cls, rng, pr, pat_widths)
    nc = _CACHE[key]

    if pats:
        masks_np = np.ascontiguousarray(
            np.concatenate(pats, axis=1).astype(np.float16))
    else:
        masks_np = np.zeros((128, 1), np.float16)

    # per-batch transposed fp8 inputs
    qT8 = [np.ascontiguousarray(q[b].T).astype(npf8) for b in range(B)]
    kT8 = [np.ascontiguousarray(k[b].T).astype(npf8) for b in range(B)]
    vT8 = [np.ascontiguousarray(v[b].T).astype(npf8) for b in range(B)]

    def pack_w3(hg):
        el = slice(hg * EL, (hg + 1) * EL)
        ws = []
        for w, sc in ((wq, 16.0), (wk, 16.0), (wv, 1.0)):
            wt = np.ascontiguousarray(w[el, :].T * sc)    # [D, EL]
            ws.append(wt.reshape(NDT, 128, EL).transpose(1, 0, 2))
        return np.ascontiguousarray(
            np.stack(ws, axis=1).reshape(128, 3 * NDT * EL)).astype(npf8)

    in_maps = []
    for c in range(NC):
        b, hg = c // 4, c % 4
        el = slice(hg * EL, (hg + 1) * EL)
        woTl = np.ascontiguousarray(wo[:, el].T.astype(np.float16))
        m = {
            "qT": qT8[b], "kT": kT8[b], "vT": vT8[b],
            "w3": pack_w3(hg),
            "woT": np.ascontiguousarray(
                woTl.reshape(2, 128, D).transpose(1, 0, 2)
                .reshape(128, 2 * D)),
            "bq": np.ascontiguousarray(
                (bq[el] * 16.0).reshape(2, 128).T),
            "masks": masks_np,
        }
        in_maps.append(m)

    res = run_bass_kernel_spmd(nc, in_maps, list(range(NC)))
    accs = []
    for b in range(B):
        acc = res.results[b * 4]["out"].astype(np.float32)
        for hg in range(1, 4):
            acc = acc + res.results[b * 4 + hg]["out"]
        accs.append(acc)
    outf = np.stack(accs).reshape(B, S, D)
    # bo plus the folded V bias: softmax weights sum to 1 so the V bias
    # contributes wo @ bv to every output row
    outf = outf + (bo + wo @ bv)[None, None, :]
    return outf


# revision 26
# speedup vs baseline: 1.0282x; 1.0034x over previous
# Trainium2 Bass kernel for nn_MultiHeadAttention (B=2, S=2048, D=1024, H=16).
#
# Sharding: batch+head tensor-parallel over 8 cores. Core c handles batch
# c//4 and head-group c%4 (4 heads, 256 e-dims): column-sharded wq/wk/wv,
# row-sharded wo with the partial-output sum done on the host. Each core
# only reads its batch's q/k/v (12MB fp16 -> 6MB fp8 per core) and writes
# a [2048, 1024] fp16 partial.
#
# Projections run as fp8e4 DoubleRow matmuls (2 contraction tiles of 128
# per pass, 0.5 cyc/row): q/k/v stream fp8 from the host, wq/wk (and bq)
# pre-scaled by 16 so the fp8 weights stay clear of subnormals; the 16*16
# score scaling is folded into the softmax exp scale (0.125/256).
#
# Attention stays fp16 (DoubleRow dst must start at partition 0, which
# makes the Z ones-row trick impossible in DR mode):
#   QT/KT = (e 128, 2 e-tiles, 2048) computed via DR with weight tiles
#   stationary; V in natural (token, e) layout packed [V_h | ones] per
#   (kt, head) so AV yields row sums (Z) free in psum row 64.
#   ScoresT = (k, q) per head; exp PSUM->SBUF on ACT gives P^T directly.
#   Causal-boundary blocks multiply P^T with a resident 0/1 triangle tile
#   on DVE (fp16 2x mode).
# Bias handling: K bias dropped (softmax per-query shift invariance),
#   V bias folded into the host-side output bias (out += wo @ bv),
#   Q bias added on DVE during the PSUM->SBUF copy.
# Host sums the 4 partial outputs per batch in fp32 and adds bo + wo@bv.

import numpy as np

B, S, D, H = 2, 2048, 1024, 16
DK = D // H            # 64
NC = 8                 # cores
NH = 4                 # heads per core
EL = NH * DK           # 256 local e-dims
NCH = 4                # projection token-chunks per core
CH = S // NCH          # 512
NDT = D // 128         # 8 contraction tiles
NDP = NDT // 2         # 4 DoubleRow contraction pairs
NKT = S // 128         # 16 k-tiles
NQB = S // 512         # 4 q-blocks

SKIP, PLAIN = -1, -2   # block classes (>=0 means partial-pattern index)


def _classify_mask(mask):
    """Per (kt, qj) block classification of the (S_q, S_k) mask.

    Returns cls[kt][qj] (SKIP / PLAIN / pattern idx), rng[kt][qj] live col
    range, pr[kt][qj] partial col range, and the deduped 0/1 patterns
    (list of [128, w] float16 arrays) for the partial ranges."""
    m = np.asarray(mask).reshape(S, S)              # [q, k]; 0 = masked
    liveT = (m != 0).T                              # [k, q]
    cls = [[PLAIN] * NQB for _ in range(NKT)]
    rng = [[(0, 512)] * NQB for _ in range(NKT)]
    pr = [[(0, 0)] * NQB for _ in range(NKT)]
    uniq = {}
    pats = []
    for kt in range(NKT):
        for qj in range(NQB):
            blk = liveT[kt * 128:(kt + 1) * 128, qj * 512:(qj + 1) * 512]
            if blk.all():
                cls[kt][qj] = PLAIN
            elif not blk.any():
                cls[kt][qj] = SKIP
            else:
                live_col = blk.any(axis=0)
                nz = np.nonzero(live_col)[0]
                c0, c1 = int(nz[0]), int(nz[-1]) + 1
                rng[kt][qj] = (c0, c1)
                part_col = live_col & ~blk.all(axis=0)
                pz = np.nonzero(part_col)[0]
                p0, p1 = int(pz[0]), int(pz[-1]) + 1
                pr[kt][qj] = (p0, p1)
                pat = blk[:, p0:p1].astype(np.float16)
                key = (p1 - p0, pat.tobytes())
                if key not in uniq:
                    uniq[key] = len(pats)
                    pats.append(np.ascontiguousarray(pat))
                cls[kt][qj] = uniq[key]
    return cls, rng, pr, pats


def _build_program(cls, rng, pr, pat_widths):
    import concourse.bacc as bacc
    import concourse.mybir as mybir
    from concourse.tile import TileContext

    f32 = mybir.dt.float32
    f16 = mybir.dt.float16
    f8 = mybir.dt.float8e4
    DR = mybir.MatmulPerfMode.DoubleRow
    Exp = mybir.ActivationFunctionType.Exp
    mult = mybir.AluOpType.mult
    ESC = 0.125 / 256.0    # exp scale: 1/sqrt(DK) with the 16*16 w-scale

    # pattern offsets inside the resident mask tile
    moff = []
    o = 0
    for w in pat_widths:
        moff.append(o)
        o += w
    MW = max(o, 1)

    nc = bacc.Bacc("TRN2", target_bir_lowering=False, debug=False,
                   num_devices=NC)

    qT = nc.dram_tensor("qT", [D, S], f8, kind="ExternalInput")
    kT = nc.dram_tensor("kT", [D, S], f8, kind="ExternalInput")
    vT = nc.dram_tensor("vT", [D, S], f8, kind="ExternalInput")
    w3d = nc.dram_tensor("w3", [128, 3 * NDT * EL], f8,
                         kind="ExternalInput")
    woT = nc.dram_tensor("woT", [128, 2 * D], f16, kind="ExternalInput")
    bqd = nc.dram_tensor("bq", [128, 2], f32, kind="ExternalInput")
    maskd = nc.dram_tensor("masks", [128, MW], f16, kind="ExternalInput")
    out = nc.dram_tensor("out", [S, D], f16, kind="ExternalOutput")

    # transposed-input views: [p, t, c] with t the 128-row d-block
    qT_r = qT.ap().rearrange("(t p) c -> p t c", p=128)
    kT_r = kT.ap().rearrange("(t p) c -> p t c", p=128)
    vT_r = vT.ap().rearrange("(t p) c -> p t c", p=128)
    w3_r = w3d.ap().rearrange("p (j t e) -> p j t e", j=3, t=NDT)
    woT_r = woT.ap().rearrange("p (i e) -> p i e", i=2)

    with TileContext(nc) as tc:
        with (
            tc.tile_pool(name="const", bufs=1) as constp,
            tc.tile_pool(name="per", bufs=1) as perp,
            tc.tile_pool(name="stage", bufs=6) as stagep,
            tc.tile_pool(name="pt", bufs=10) as ptp,
            tc.tile_pool(name="zz", bufs=4) as zzp,
            tc.tile_pool(name="zb", bufs=8) as zbp,
            tc.tile_pool(name="ost", bufs=6) as ostp,
            tc.tile_pool(name="psA", bufs=2, space="PSUM") as psA,
            tc.tile_pool(name="psS", bufs=2, space="PSUM") as psS,
            tc.tile_pool(name="psO", bufs=2, space="PSUM") as psO,
        ):
            # ---- constants. w3 arrives per-tensor, interleaved with the
            # input staging DMAs, so the first matmuls start early ----
            w3 = constp.tile([128, 3, NDT, EL], f8, tag="w3")
            bq_sb = constp.tile([128, 2], f32, tag="bq")
            msk = constp.tile([128, MW], f16, tag="msk")
            woT_sb = constp.tile([128, 2, D], f16, tag="wo")

            # ---- persistent activations ----
            # QT/KT/OT: [p, e-tile, token]; head h -> (rows (h%2)*64,
            # e-tile h//2)
            QT_sb = perp.tile([128, 2, S], f16, tag="QT")
            KT_sb = perp.tile([128, 2, S], f16, tag="KT")
            OT_sb = perp.tile([128, 2, S], f16, tag="OT")
            # V natural (token, e) packed per (kt, head) as
            # [ones(64, last col=1) | V_h(64)] so the AV stationary
            # [ones-col | V] is a contiguous 65-col AP and Z (the exp row
            # sum) lands in psum row 0 -- partition 0, where the DVE
            # reciprocal can read it without a staging copy.
            V_big = perp.tile([128, NKT * NH * 2 * 64], f16, tag="Vb")
            V3 = V_big[:].rearrange("p (t x) -> p t x", x=64)

            def emit_vz():
                nc.vector.memset(V3[:, 0::2, :], 0.0)
                nc.vector.memset(V3[:, 0::2, 63:64], 1.0)

            # ---- projections (fp8 DoubleRow, 2 d-tiles per pass) ----
            # generators yield after each PE quantum (~0.2-0.4us) so the
            # attention loop can drain them as PE filler between groups
            JSRC = {"q": (0, qT_r), "k": (1, kT_r), "v": (2, vT_r)}

            def stage_tensor(name, c, split=False):
                j, src_r = JSRC[name]
                lo = c * CH
                st = stagep.tile([128, NDT, CH], f8, tag="stage")
                if split:
                    # land the first two d-tiles early so the PE starts
                    # before the full chunk arrives
                    nc.sync.dma_start(out=st[:, 0:2, :],
                                      in_=src_r[:, 0:2, lo:lo + CH])
                    nc.sync.dma_start(out=st[:, 2:NDT, :],
                                      in_=src_r[:, 2:NDT, lo:lo + CH])
                else:
                    nc.sync.dma_start(out=st[:], in_=src_r[:, :, lo:lo + CH])
                return st

            def gen_qk_ep(name, c, ep, st):
                # transposed layout [e, token]; out [64 e, 512] per piece;
                # psum [128, 512] holds 2 pieces (bases 0, 64)
                w = w3[:, JSRC[name][0]]
                lo = c * CH
                ps = psA.tile([128, 512], f32, tag="proj",
                              name=f"{name}p{c}{ep}")
                for pp in range(2):
                    sub = ps[pp * 64:pp * 64 + 64, :]
                    e0 = ep * 128 + pp * 64
                    for t in range(NDP):
                        nc.tensor.matmul(
                            sub, w[:, 2 * t:2 * t + 2, e0:e0 + 64],
                            st[:, 2 * t:2 * t + 2, :],
                            start=(t == 0), stop=(t == NDP - 1),
                            perf_mode=DR)
                    yield
                if name == "q":
                    nc.vector.tensor_scalar_add(
                        QT_sb[:, ep, lo:lo + CH], ps[:], bq_sb[:, ep:ep + 1])
                else:
                    nc.vector.tensor_copy(KT_sb[:, ep, lo:lo + CH], ps[:])

            def gen_v(c, st):
                # natural layout: out [64 tok, 256 e] per subtile; a psum
                # [128, 512] holds one kt (128 tokens) per col-half
                w = w3[:, 2]
                for half in range(2):
                    ps = psA.tile([128, 512], f32, tag="proj",
                                  name=f"vp{c}{half}")
                    for pos in range(4):       # (base, col-half)
                        t0 = half * 256 + pos * 64
                        ob = (pos % 2) * 64
                        oc = (pos // 2) * 256
                        sub = ps[ob:ob + 64, oc:oc + 256]
                        for t in range(NDP):
                            nc.tensor.matmul(
                                sub, st[:, 2 * t:2 * t + 2, t0:t0 + 64],
                                w[:, 2 * t:2 * t + 2, :],
                                start=(t == 0), stop=(t == NDP - 1),
                                perf_mode=DR)
                        if pos == 1:
                            yield
                    for kk in range(2):        # kt within this psum tile
                        kt = c * 4 + half * 2 + kk
                        nc.vector.tensor_copy(
                            V3[:, kt * NH * 2 + 1:(kt + 1) * NH * 2:2, :],
                            ps[:, kk * 256:(kk + 1) * 256]
                            .rearrange("p (h e) -> p h e", h=NH))
                    yield

            def gen_chunk3(c):
                # e-tile 0 of q AND k first: the next q-block's pair-A
                # scores need exactly those
                stq = stage_tensor("q", c)
                stk = stage_tensor("k", c)
                yield from gen_qk_ep("q", c, 0, stq)
                yield from gen_qk_ep("k", c, 0, stk)
                yield from gen_qk_ep("q", c, 1, stq)
                yield from gen_qk_ep("k", c, 1, stk)
                stv = stage_tensor("v", c)
                yield from gen_v(c, stv)

            def chain(*gens):
                for g in gens:
                    yield from g

            def drain(g):
                for _ in g:
                    pass

            # ---- attention ----
            # Per q-block, heads processed as two interleaved pairs so PE
            # always has the other head's matmuls while ACT runs exp. The
            # AV matmul lags scores by one group so it never waits on exp,
            # and a caller-supplied filler (projection / oproj pieces) is
            # drained between groups to keep the PE queue dense.
            def emit_attention(qj, pairs=(0, 1), defer_oproj=False,
                               tail=False, kw=False, filler=None, pulls=1):
                qlo = qj * 512
                acts = [kt for kt in range(NKT) if cls[kt][qj] != SKIP]
                if not acts:
                    return
                groups = [acts[gi:gi + 2] for gi in range(0, len(acts), 2)]

                def pull_filler(n):
                    for _ in range(n):
                        if filler is not None:
                            try:
                                next(filler)
                                continue
                            except StopIteration:
                                pass
                        if kw:
                            emit_keepwarm(1, 700 + qj * 100
                                          + pull_filler.kwn)
                            pull_filler.kwn += 1
                        break
                pull_filler.kwn = 0

                def emit_scores(grp, h, sc):
                    ep, hp = h // 2, (h % 2) * 64
                    for i, kt in enumerate(grp):
                        c0, c1 = rng[kt][qj]
                        klo = kt * 128
                        nc.tensor.matmul(
                            sc[:, i * 512 + c0:i * 512 + c1],
                            KT_sb[hp:hp + 64, ep, klo:klo + 128],
                            QT_sb[hp:hp + 64, ep, qlo + c0:qlo + c1],
                            start=True, stop=True)

                def emit_exp_mask(grp, pt, sc):
                    spans = [(i * 512 + rng[kt][qj][0],
                              i * 512 + rng[kt][qj][1])
                             for i, kt in enumerate(grp)]
                    lo, hi2 = spans[0][0], spans[-1][1]
                    dead = (hi2 - lo) - sum(b - a for a, b in spans)
                    exp_spans = spans if dead > 200 else [(lo, hi2)]
                    for a, bnd in exp_spans:
                        nc.scalar.activation(pt[:, a:bnd], sc[:, a:bnd],
                                             Exp, scale=ESC)
                    for i, kt in enumerate(grp):
                        cl = cls[kt][qj]
                        if cl >= 0:
                            pp0, pp1 = pr[kt][qj]
                            sl = slice(i * 512 + pp0, i * 512 + pp1)
                            nc.vector.tensor_tensor(
                                pt[:, sl], pt[:, sl],
                                msk[:, moff[cl]:moff[cl] + pp1 - pp0],
                                op=mult)

                def emit_av(grp, hi, h, pt, ots, n_done):
                    for i, kt in enumerate(grp):
                        c0, c1 = rng[kt][qj]
                        base = (kt * NH + h) * 128
                        vap = V_big[:, base + 63:base + 128]
                        n_done[hi] += 1
                        nc.tensor.matmul(
                            ots[hi][0:65, c0:c1], vap,
                            pt[:, i * 512 + c0:i * 512 + c1],
                            start=(n_done[hi] == 1),
                            stop=(n_done[hi] == len(acts)))

                def emit_norm(pair, hi, h, ots):
                    # normalize: row 0 of ot = Z (the ones column). 65-row
                    # copy moves Z+AV to SBUF and frees the PSUM tile
                    # early; the reciprocal reads Z at partition 0 with no
                    # staging copy.
                    ep, hp = h // 2, (h % 2) * 64
                    ot = ots[hi]
                    oc = zbp.tile([65, 512], f32, tag="oc")
                    if tail and pair == pairs[-1] and hi == 0:
                        # ACT is idle once the last exps are done
                        nc.scalar.copy(oc[:], ot[0:65, :])
                    else:
                        nc.vector.tensor_copy(oc[:], ot[0:65, :])
                    rz = zzp.tile([1, 512], f32, tag="z")
                    nc.vector.reciprocal_approx_fast(rz[:], oc[0:1, :])
                    rb = zbp.tile([64, 512], f32, tag="zb")
                    nc.gpsimd.partition_broadcast(rb[:], rz[:],
                                                  channels=64)
                    nc.vector.tensor_tensor(
                        OT_sb[hp:hp + 64, ep, qlo:qlo + 512],
                        oc[1:65, :], rb[:], op=mult)

                # Flat item stream; with two pairs, pair B's first group is
                # hoisted into pair A's last so the exp pipeline never
                # drains at the pair boundary.
                def pair_items(pair):
                    return [(pair, grp, hi) for grp in groups
                            for hi in range(2)]

                if len(pairs) == 2:
                    a, bb = pair_items(pairs[0]), pair_items(pairs[1])
                    stream = a[:-2]
                    for x, y in zip(bb[:2], a[-2:]):
                        stream += [x, y]
                    stream += bb[2:]
                else:
                    stream = pair_items(pairs[0])

                ots = {}
                n_done = {}
                n_av = {p: 0 for p in pairs}
                pend = []   # (pair, grp, hi, h, pt): AV lags one item

                def flush_av():
                    pair, grp, hi, h, pt = pend.pop(0)
                    emit_av(grp, hi, h, pt, ots[pair], n_done[pair])
                    n_av[pair] += len(grp)
                    if n_av[pair] == 2 * len(acts):
                        for hj in range(2):
                            emit_norm(pair, hj, pair * 2 + hj, ots[pair])

                for idx, (pair, grp, hi) in enumerate(stream):
                    if pair not in ots:
                        ots[pair] = [psO.tile([128, 512], f32, tag="ot",
                                              name=f"ot{qj}{pair * 2 + j}")
                                     for j in range(2)]
                        n_done[pair] = [0, 0]
                    h = pair * 2 + hi
                    sc = psS.tile([128, 1024], f32, tag="score")
                    emit_scores(grp, h, sc)
                    pt = ptp.tile([128, 1024], f16, tag="pt")
                    emit_exp_mask(grp, pt, sc)
                    if pend:
                        flush_av()
                    pend.append((pair, grp, hi, h, pt))
                    if kw and idx >= len(stream) - 6:
                        pass    # keep the critical tail free of filler
                    else:
                        pull_filler(pulls)
                while pend:
                    flush_av()
                if not defer_oproj:
                    emit_oproj_qblock(qj)

            # ---- output projection (partial over local e-dims) ----
            # generator yielding per half-tile; psA ring (transient, like
            # the projection pieces it interleaves with)
            def gen_oproj(qj, act_copy=False):
                for g in range(qj * 4, (qj + 1) * 4):
                    osr = ostp.tile([128, D], f16, tag="ost")
                    for jh in range(2):
                        po = psA.tile([128, 512], f32, tag="proj",
                                      name=f"po{g}{jh}")
                        for ep in range(2):
                            nc.tensor.matmul(
                                po[:], OT_sb[:, ep, g * 128:(g + 1) * 128],
                                woT_sb[:, ep, jh * 512:(jh + 1) * 512],
                                start=(ep == 0), stop=(ep == 1))
                        if act_copy and jh == 0:
                            nc.scalar.copy(
                                osr[:, jh * 512:(jh + 1) * 512], po[:])
                        else:
                            nc.vector.tensor_copy(
                                osr[:, jh * 512:(jh + 1) * 512], po[:])
                        if act_copy:
                            # tail: stream each half out as soon as its
                            # copy lands instead of waiting for the pair
                            nc.sync.dma_start(
                                out=out.ap()[g * 128:(g + 1) * 128,
                                             jh * 512:(jh + 1) * 512],
                                in_=osr[:, jh * 512:(jh + 1) * 512])
                        yield
                    if not act_copy:
                        nc.sync.dma_start(
                            out=out.ap()[g * 128:(g + 1) * 128, :],
                            in_=osr[:])

            def emit_oproj_qblock(qj, tail=False):
                drain(gen_oproj(qj, act_copy=tail))

            # keep-warm: tiny write-only matmuls fill PE idle so following
            # matmuls run at full clock instead of re-ramping.
            def emit_keepwarm(n, tag0):
                for i in range(n):
                    kwt = psA.tile([128, 512], f32, tag="proj",
                                   name=f"kw{tag0}{i}")
                    nc.tensor.matmul(kwt[:, 0:256], woT_sb[:, 0, 0:128],
                                     woT_sb[:, 0, 0:256],
                                     start=True, stop=True)

            # ---- schedule ----
            # Startup: q/k e-tile 0 of chunk 0 projects first so qj0 pair A
            # attention starts as early as possible; the rest of chunk 0
            # fills its PE gaps. Then qj order [0B, 1, 2, 3] with the next
            # projection chunk and the previous q-block's oproj interleaved
            # into each attention loop as PE filler. The tail is qj3's last
            # norms + its oproj, streamed out per half-tile.
            emit_vz()
            # startup DMA order: q/k first pieces and their weights lead;
            # v and the o-projection weights follow the critical path
            nc.sync.dma_start(out=w3[:, 0], in_=w3_r[:, 0])
            nc.scalar.dma_start(out=bq_sb[:], in_=bqd.ap()[:])
            lo0 = 0
            stq0 = stagep.tile([128, NDT, CH], f8, tag="stage")
            nc.sync.dma_start(out=stq0[:, 0:2, :], in_=qT_r[:, 0:2, 0:CH])
            nc.sync.dma_start(out=w3[:, 1], in_=w3_r[:, 1])
            stk0 = stagep.tile([128, NDT, CH], f8, tag="stage")
            nc.sync.dma_start(out=stk0[:, 0:2, :], in_=kT_r[:, 0:2, 0:CH])
            nc.sync.dma_start(out=stq0[:, 2:NDT, :],
                              in_=qT_r[:, 2:NDT, 0:CH])
            nc.scalar.dma_start(out=msk[:], in_=maskd.ap()[:])
            nc.sync.dma_start(out=stk0[:, 2:NDT, :],
                              in_=kT_r[:, 2:NDT, 0:CH])
            drain(gen_qk_ep("q", 0, 0, stq0))
            drain(gen_qk_ep("k", 0, 0, stk0))
            nc.sync.dma_start(out=w3[:, 2], in_=w3_r[:, 2])
            stv0 = stage_tensor("v", 0)
            g0rest = chain(gen_v(0, stv0),
                           gen_qk_ep("q", 0, 1, stq0),
                           gen_qk_ep("k", 0, 1, stk0))
            emit_attention(0, pairs=(0,), defer_oproj=True, filler=g0rest)
            drain(g0rest)
            nc.scalar.dma_start(out=woT_sb[:], in_=woT_r[:])
            g1 = gen_chunk3(1)
            emit_attention(0, pairs=(1,), defer_oproj=True, filler=g1,
                           pulls=4)
            drain(g1)
            g2 = chain(gen_chunk3(2), gen_oproj(0))
            emit_attention(1, defer_oproj=True, filler=g2, pulls=2)
            drain(g2)
            g3 = chain(gen_chunk3(3), gen_oproj(1))
            emit_attention(2, defer_oproj=True, filler=g3)
            drain(g3)
            emit_attention(3, defer_oproj=True, tail=True, kw=True,
                           filler=gen_oproj(2))
            # ramp the PE back up while the last norms run on DVE/Pool so
            # the tail oproj matmuls execute at full clock
            emit_keepwarm(10, 900)
            emit_oproj_qblock(3, tail=True)

    nc.compile()
    return nc


_CACHE = {}


def kernel(q, k, v, mask, wq, bq, wk, bk, wv, bv, wo, bo):
    import ml_dtypes
    from concourse.bass_utils import run_bass_kernel_spmd
    npf8 = ml_dtypes.float8_e4m3

    q = np.asarray(q, np.float32)
    k = np.asarray(k, np.float32)
    v = np.asarray(v, np.float32)
    wq = np.asarray(wq, np.float32)
    wk = np.asarray(wk, np.float32)
    wv = np.asarray(wv, np.float32)
    wo = np.asarray(wo, np.float32)
    bq = np.asarray(bq, np.float32)
    bv = np.asarray(bv, np.float32)
    bo = np.asarray(bo, np.float32)

    cls, rng, pr, pats = _classify_mask(mask)
    pat_widths = [p.shape[1] for p in pats]
    key = (tuple(tuple(r) for r in cls), tuple(tuple(r) for r in rng),
           tuple(tuple(r) for r in pr), tuple(pat_widths))
    if key not in _CACHE:
        _CACHE[key] = _build_program(cls, rng, pr, pat_widths)
    nc = _CACHE[key]

    if pats:
        masks_np = np.ascontiguousarray(
            np.concatenate(pats, axis=1).astype(np.float16))
    else:
        masks_np = np.zeros((128, 1), np.float16)

    # per-batch transposed fp8 inputs
    qT8 = [np.ascontiguousarray(q[b].T).astype(npf8) for b in range(B)]
    kT8 = [np.ascontiguousarray(k[b].T).astype(npf8) for b in range(B)]
    vT8 = [np.ascontiguousarray(v[b].T).astype(npf8) for b in range(B)]

    def pack_w3(hg):
        el = slice(hg * EL, (hg + 1) * EL)
        ws = []
        for w, sc in ((wq, 16.0), (wk, 16.0), (wv, 1.0)):
            wt = np.ascontiguousarray(w[el, :].T * sc)    # [D, EL]
            ws.append(wt.reshape(NDT, 128, EL).transpose(1, 0, 2))
        return np.ascontiguousarray(
            np.stack(ws, axis=1).reshape(128, 3 * NDT * EL)).astype(npf8)

    in_maps = []
    for c in range(NC):
        b, hg = c // 4, c % 4
        el = slice(hg * EL, (hg + 1) * EL)
        woTl = np.ascontiguousarray(wo[:, el].T.astype(np.float16))
        m = {
            "qT": qT8[b], "kT": kT8[b], "vT": vT8[b],
            "w3": pack_w3(hg),
            "woT": np.ascontiguousarray(
                woTl.reshape(2, 128, D).transpose(1, 0, 2)
                .reshape(128, 2 * D)),
            "bq": np.ascontiguousarray(
                (bq[el] * 16.0).reshape(2, 128).T),
            "masks": masks_np,
        }
        in_maps.append(m)

    res = run_bass_kernel_spmd(nc, in_maps, list(range(NC)))
    accs = []
    for b in range(B):
        acc = res.results[b * 4]["out"].astype(np.float32)
        for hg in range(1, 4):
            acc = acc + res.results[b * 4 + hg]["out"]
        accs.append(acc)
    outf = np.stack(accs).reshape(B, S, D)
    # bo plus the folded V bias: softmax weights sum to 1 so the V bias
    # contributes wo @ bv to every output row
    outf = outf + (bo + wo @ bv)[None, None, :]
    return outf
